# revision 40
# baseline (speedup 1.0000x reference)
"""Trainium2 Bass kernel for DimSpecializedAttention.

Problem: B=8, T=2048, D=1280, H=10 heads, head_dim=128.
  q/k/v = x @ W{q,k,v}.T ; RoPE(q, k) ; causal softmax(q k^T / sqrt(128));
  per-head sigmoid gate (from consciousness_vector) applied post-softmax;
  out = (att @ v) @ Wo.T

Sharding: data-parallel over batch — core b gets batch b (8 cores, B=8).

Per-core kernel design (all matmuls bf16 with fp32 PSUM accumulation):
  - projections computed in transposed layout qT/kT [e, t] so the head dim
    lands on partitions (contraction-ready for attention); v in [t, e]
    layout with a ones-column appended per head ("vaug", stride 129).
  - scores computed transposed: S^T[tk, tq] = kT_j^T @ qT, causal blocks
    only (tq >= 128*j), exp on ScalarE straight out of PSUM (no
    max-subtraction: scores are ~N(0,1), max << 80, fp32-safe).
  - PV uses P^T tiles as the stationary operand and [v_j | 1] as moving:
    out[tq, 0:128] = attention numerator, out[:, 128] = softmax
    denominator — one fused accumulation chain per 128-row query block.
  - rows scaled by gate_h / denom (DVE), written to y; per 512-query
    group the output projection (PE transpose of y + Wo matmuls) runs
    overlapped with the next group's attention.
"""

import numpy as np
import ml_dtypes

BF16 = ml_dtypes.bfloat16

B, T, D = 8, 2048, 1280
H, HD = 10, 128
NCORES = 8
DC = D // 128      # 10 d-chunks
TB = T // 128      # 16 t-blocks
NG = 4             # attention groups of 512 queries
SCALE = float(1.0 / np.sqrt(HD))
VW = HD + 1        # 129: v columns per head incl. ones column

_cache = {}


def _build_program():
    import os
    import concourse.bacc as bacc
    import concourse.mybir as mybir
    import concourse.tile as tile
    from concourse.tile_rust import add_dep_helper
    from contextlib import ExitStack
    from collections import deque

    stage = os.environ.get("KSTAGE", "full")  # debug: proj | attn | full

    f32 = mybir.dt.float32
    bf16 = mybir.dt.bfloat16
    MUL = mybir.AluOpType.mult
    EXP = mybir.ActivationFunctionType.Exp

    nc = bacc.Bacc("TRN2", target_bir_lowering=False, debug=False,
                   num_devices=NCORES)

    xt_d = nc.dram_tensor("xt", [128, DC * T], bf16, kind="ExternalInput")
    wq_d = nc.dram_tensor("wq", [128, DC * D], bf16, kind="ExternalInput")
    wk_d = nc.dram_tensor("wk", [128, DC * D], bf16, kind="ExternalInput")
    wv_d = nc.dram_tensor("wv", [128, DC * D], bf16, kind="ExternalInput")
    wo_d = nc.dram_tensor("wo", [128, H * D], bf16, kind="ExternalInput")
    cos_d = nc.dram_tensor("cosr", [128, T], bf16, kind="ExternalInput")
    srot_d = nc.dram_tensor("srot", [128, T], bf16, kind="ExternalInput")
    mask_d = nc.dram_tensor("trimask", [128, 128], bf16, kind="ExternalInput")
    ident_d = nc.dram_tensor("ident", [128, 128], bf16, kind="ExternalInput")
    # bf16 output: halves the out-DMA bytes (tail latency); host upcasts.
    out_d = nc.dram_tensor("out", [T, D], bf16, kind="ExternalOutput")

    with tile.TileContext(nc) as tc, ExitStack() as ctx:
        # ---- persistent pools -------------------------------------------
        pool_const = ctx.enter_context(tc.tile_pool(name="const", bufs=1))
        pool_qkv = ctx.enter_context(tc.tile_pool(name="qkv", bufs=1))
        # xt quarter 1 and wv's n0=1024 piece live in persistent pools:
        # the v-projection chains for (tb 4-7, heads 8-9) are DEFERRED
        # into group 0's attention as PE filler (g0 is ACT/DVE-bound
        # with ~10us of PE idle and no o-proj available by causality),
        # so their inputs must survive into phase 2.
        pool_xt1 = ctx.enter_context(tc.tile_pool(name="xtq1", bufs=1))
        pool_wv3 = ctx.enter_context(tc.tile_pool(name="wv3", bufs=1))

        mask_t = pool_const.tile([128, 128], bf16, tag="mask")
        ident_t = pool_const.tile([128, 128], bf16, tag="ident")

        qt = pool_qkv.tile([128, H * T], bf16, tag="qt")
        kt = pool_qkv.tile([128, H * T], bf16, tag="kt")
        vaug = pool_qkv.tile([128, TB * VW * H], bf16, tag="vaug")
        # eb=0's wq block preloaded on the sync ring: the rotating web
        # pool aliases wv's range, so its first DMA is WAR-gated behind
        # the whole v-projection — this tile isn't.
        web0_t = pool_qkv.tile([128, D], bf16, tag="web0")

        # ---- phase 1: projections ---------------------------------------
        with tc.tile_pool(name="xtp", bufs=1) as pool_xt, \
             tc.tile_pool(name="projtmp", bufs=6) as pool_ptmp, \
             tc.tile_pool(name="projpsum", bufs=7, space="PSUM") as psum_proj:

            # xt is laid out quarter-major: col = q*DC*512 + c*512 + tl
            # (t = 512q + tl). Each t-quarter is one contiguous column
            # range, so the four loads are full-rate AND the dependency
            # tracker sees disjoint ranges — the first v chains start as
            # soon as quarter 0 lands (~19us) instead of after the whole
            # 5MB transfer.
            QW = DC * 512
            # quarters 0,2,3 in a phase-1 tile; quarter 1 persistent
            # (the deferred v-tail chains read it during attention g0)
            xt_r = pool_xt.tile([128, 3 * QW], bf16, tag="xt")
            xt_q1 = pool_xt1.tile([128, QW], bf16, tag="xtq1")

            def xt_at(q, off, w):
                if q == 1:
                    return xt_q1[:, off:off + w]
                base = (0, None, QW, 2 * QW)[q]
                return xt_r[:, base + off:base + off + w]

            # eighth-granular pieces, split across BOTH DMA rings so
            # arrival (~1.9us each) stays ahead of the v-proj consumption
            # rate (~2.1us per t-block chain): pieces 0-3 here on sync;
            # pieces 4-7 ride the scalar ring, emitted after wv's first
            # piece in the v-proj section below.
            for p0, pw in ((0, QW // 4), (QW // 4, QW // 4),
                           (QW // 2, QW // 2)):
                nc.sync.dma_start(xt_r[:, p0:p0 + pw],
                                  xt_d[:, p0:p0 + pw])
            for p0, pw in ((QW, QW // 2), (3 * QW // 2, QW // 2)):
                nc.sync.dma_start(xt_q1[:, p0 - QW:p0 - QW + pw],
                                  xt_d[:, p0:p0 + pw])
            nc.sync.dma_start(web0_t[:], wq_d[:, 0:D])
            # mask/ident after the xt pieces: they aren't read until the
            # proj-end bridge / attention, and descriptor-issue time at
            # the queue head delays the first xt piece ~1.6us otherwise
            nc.sync.dma_start(mask_t[:], mask_d[:])
            nc.sync.dma_start(ident_t[:], ident_d[:])

            # HAM warm-up bridge: keep the PE busy with throwaway matmuls
            # while the x/wv DMAs are in flight, so the real chains start
            # at the full 2.4GHz clock instead of the cold 1.2GHz default.
            # Tiles come from the long-lived proj pools — a dedicated pool
            # would alias the wv tile's address range and stall its DMA
            # until the last warm-up matmul retires.
            wt = pool_ptmp.tile([128, 512], bf16, tag="t2", name="warm_t")
            nc.gpsimd.memset(wt[:], 0.0)
            wps = psum_proj.tile([128, 512], f32, tag="pp", name="warm_ps")
            for _ in range(12):
                nc.tensor.matmul(wps[:], wt[:, 0:128], wt[:],
                                 start=True, stop=True,
                                 skip_group_check=True)
            # extend HAM-warm coverage deeper into the xt DMA wait with
            # N=256 warmups (trimmed to match the faster first-chain DMA
            # path: xt piece 0 + wv c-chunk 0 land ~18-19us)
            for _ in range(6):
                nc.tensor.matmul(wps[:, 0:256], wt[:, 0:128],
                                 wt[:, 0:256], start=True, stop=True,
                                 skip_group_check=True)

            # vaug ones: memset ONLY each head's 129th column (160 elems
            # per partition, strided). The old full-tile memset took 18us
            # and its WAW dep stalled the v-proj copies ~6us at startup.
            nc.gpsimd.memset(
                vaug[:].rearrange("p (tb h c) -> p tb h c",
                                  tb=TB, h=H)[:, :, :, 128:129], 1.0)

            # v projection into vaug (head-interleaved, stride VW) — first,
            # so attention's PV inputs are ready as early as possible.
            with tc.tile_pool(name="wvp", bufs=1) as pool_wv:
                wv_t = pool_wv.tile([128, DC * D], bf16, tag="wv")
                # wv is pre-permuted host-side so each e-range piece is a
                # CONTIGUOUS column range: strided-piece DMAs ran at only
                # ~37GB/s and hogged the scalar ring for ~100us, starving
                # web/wo behind them (5.3us q/k stall + 6.7us boundary
                # stall while the wo gate-fold waited).
                # First piece fine-grained at the head: the first v
                # chain's c=0 matmul needs only cols [0,512) (131KB), so
                # it starts as soon as that lands; later c-chunks stream
                # in ahead of the chain's consumption.
                for s0, s1 in ((0, 512), (512, 1536), (1536, 2560),
                               (2560, DC * 512)):
                    nc.scalar.dma_start(wv_t[:, s0:s1], wv_d[:, s0:s1])
                # xt pieces 4-7 on the scalar ring, behind wv's first
                # piece (needed first) but ahead of its later ones
                for piece in range(4, 8):
                    p0 = piece * (QW // 2)
                    csl = slice(p0, p0 + QW // 2)
                    nc.scalar.dma_start(
                        xt_r[:, p0 - QW:p0 - QW + QW // 2], xt_d[:, csl])
                nc.scalar.dma_start(wv_t[:, DC * 512:2 * DC * 512],
                                    wv_d[:, DC * 512:2 * DC * 512])
                wv3_t = pool_wv3.tile([128, DC * 256], bf16, tag="wv3")
                nc.scalar.dma_start(
                    wv3_t[:], wv_d[:, 2 * DC * 512:2 * DC * 512 + DC * 256])
                for n0, nw in ((0, 512), (512, 512), (1024, 256)):
                    for tb in range(TB):
                        if n0 == 1024 and 4 <= tb < 8:
                            continue   # deferred: g0-attention filler
                        ps = psum_proj.tile([128, 512], f32, tag="pp")
                        for c in range(DC):
                            wsrc = (wv3_t[:, c * nw:(c + 1) * nw]
                                    if n0 == 1024 else
                                    wv_t[:, (0 if n0 == 0 else DC * 512)
                                         + c * nw:
                                         (0 if n0 == 0 else DC * 512)
                                         + (c + 1) * nw])
                            nc.tensor.matmul(
                                ps[:, 0:nw],
                                xt_at(tb // 4,
                                      c * 512 + (tb % 4) * 128, 128),
                                wsrc,
                                start=(c == 0), stop=(c == DC - 1))
                        for k in range(nw // 128):
                            h = (n0 + k * 128) // 128
                            base = tb * VW * H + h * VW
                            nc.vector.tensor_copy(
                                vaug[:, base:base + 128],
                                ps[:, k * 128:(k + 1) * 128])

            # q/k projections (transposed out, + RoPE)
            with tc.tile_pool(name="web", bufs=2) as pool_web, \
                 tc.tile_pool(name="rope", bufs=1) as pool_rope:
                cos_t = pool_rope.tile([128, T], bf16, tag="cos")
                srot_t = pool_rope.tile([128, T], bf16, tag="srot")
                # sync ring: transfers in parallel with web0 on the
                # scalar ring once the wv-range WAR releases
                nc.sync.dma_start(cos_t[:], cos_d[:])
                nc.sync.dma_start(srot_t[:], srot_d[:])

                for eb in range(H):
                    for w_d, dst in ((wq_d, qt), (wk_d, kt)):
                        if eb == 0 and w_d is wq_d:
                            web = web0_t     # preloaded on the sync ring
                        else:
                            web = pool_web.tile([128, D], bf16, tag="web")
                            nc.scalar.dma_start(
                                web[:], w_d[:, eb * D:(eb + 1) * D])
                        for tcn in range(T // 512):
                            ps = psum_proj.tile([128, 512], f32, tag="pp")
                            for c in range(DC):
                                nc.tensor.matmul(
                                    ps[:],
                                    web[:, c * 128:(c + 1) * 128],
                                    xt_at(tcn, c * 512, 512),
                                    start=(c == 0), stop=(c == DC - 1))
                            # RoPE: ScalarE stages the partition-rotated
                            # copy from PSUM (ACT is idle in this phase),
                            # then DVE does mul/mul/add.
                            qrot = pool_ptmp.tile([128, 512], bf16,
                                                  tag="qrot")
                            nc.scalar.copy(qrot[0:64, :], ps[64:128, :])
                            nc.scalar.copy(qrot[64:128, :], ps[0:64, :])
                            t2 = pool_ptmp.tile([128, 512], bf16, tag="t2")
                            sl = slice(tcn * 512, (tcn + 1) * 512)
                            o = dst[:, eb * T + tcn * 512:
                                    eb * T + (tcn + 1) * 512]
                            nc.vector.tensor_mul(t2[:], qrot[:],
                                                 srot_t[:, sl])
                            nc.vector.tensor_mul(o, ps[:], cos_t[:, sl])
                            nc.vector.tensor_add(o, o, t2[:])

            wps_c = psum_proj.tile([128, 512], f32, tag="pp",
                                   name="bridge2_ps")
            for _ in range(8):
                nc.tensor.matmul(wps_c[:], ident_t[:], xt_r[:, 0:512],
                                 start=True, stop=True,
                                 skip_group_check=True)

        if stage == "proj":
            # debug: dump slices of qt/kt/vaug into out rows
            with tc.tile_pool(name="dbg", bufs=2) as pool_dbg:
                for nm, src in (("q", qt), ("k", kt), ("v", vaug)):
                    od = pool_dbg.tile([128, 1024], bf16, tag="od",
                                       name=f"od_{nm}")
                    nc.scalar.copy(od[:], src[:, 0:1024])
                    row = {"q": 0, "k": 128, "v": 256}[nm]
                    nc.sync.dma_start(out_d[row:row + 128, 0:1024], od[:])

        # ---- phase 2+3: attention + output projection -------------------
        def phase23():
          with tc.tile_pool(name="pt", bufs=6) as pool_pt, \
             tc.tile_pool(name="yg", bufs=2) as pool_y, \
             tc.tile_pool(name="yta", bufs=2) as pool_yta, \
             tc.tile_pool(name="osb", bufs=2) as pool_osb, \
             tc.tile_pool(name="wop", bufs=1) as pool_wo, \
             tc.tile_pool(name="small", bufs=8) as pool_small, \
             tc.tile_pool(name="spsum", bufs=2, space="PSUM") as psum_s, \
             tc.tile_pool(name="pvpsum", bufs=2, space="PSUM") as psum_pv, \
             tc.tile_pool(name="ytpsum", bufs=1, space="PSUM") as psum_yt, \
             tc.tile_pool(name="opsum", bufs=1, space="PSUM") as psum_o:

            # wo on the otherwise-idle SYNC ring (the scalar ring still
            # has web DMAs queued ahead at phase-2 entry, which delayed
            # wo to ~300us and stalled the first o-proj filler 6.6us),
            # in per-head pieces so the first o-proj chain (which reads
            # only head 0's columns first) can start on piece 0.
            wo_t = pool_wo.tile([128, H * D], bf16, tag="wo")
            for h_ in range(H):
                nc.sync.dma_start(wo_t[:, h_ * D:(h_ + 1) * D],
                                  wo_d[:, h_ * D:(h_ + 1) * D])

            def oproj_steps(g, ytall_g, alt=False, dve_copies=False):
                """Emit group g's output projection one instruction per
                yield, so the caller can pace it into the next group's
                strip-pair loop as PE filler for the exp-wait stalls.
                alt=True (final drain only, when the transpose PSUM bank
                is idle) alternates chains across two banks so each
                chain overlaps the previous chain's drain copy.
                dve_copies=True routes all drain copies to DVE — for
                o-proj paced into g2/g3, whose regions are ACT-bound
                (exp-saturated) while DVE has ~60% slack."""
                for p_ in range(4):
                    tb = 4 * g + p_
                    o_sb = pool_osb.tile([128, D], bf16, tag="osb")
                    for ci, (n0, nw) in enumerate(
                            ((0, 512), (512, 512), (1024, 256))):
                        k = p_ * 3 + ci
                        if alt:
                            # final drain: rotate chains over FOUR psum
                            # banks (o, yt, pva, pvb — attention is done,
                            # all free) so each chain's drain copy has 3
                            # chain-times to complete instead of 1
                            if k % 4 == 0:
                                ops = psum_o.tile([128, 512], f32,
                                                  tag="ops")
                            elif k % 4 == 1:
                                ops = psum_yt.tile([128, 512], f32,
                                                   tag="yt")
                            else:
                                ops = psum_pv.tile([128, 512], f32,
                                                   tag="pv")
                        else:
                            ops = psum_o.tile([128, 512], f32, tag="ops")
                        for h in range(H):
                            nc.tensor.matmul(
                                ops[:, 0:nw],
                                ytall_g[:, p_ * D + h * 128:
                                        p_ * D + (h + 1) * 128],
                                wo_t[:, h * D + n0:h * D + n0 + nw],
                                start=(h == 0), stop=(h == H - 1))
                            yield
                        use_scalar = ((k % 2 == 0) if alt
                                      else (ci == 1 and not dve_copies))
                        if use_scalar:
                            nc.scalar.copy(o_sb[:, n0:n0 + nw],
                                           ops[:, 0:nw])
                        else:
                            nc.vector.tensor_copy(o_sb[:, n0:n0 + nw],
                                                  ops[:, 0:nw])
                        yield
                        if alt and p_ == 3:
                            # last block: per-piece DMA right after each
                            # copy, so the final transfer tail is one
                            # 256-col piece instead of the whole block
                            dq = (nc.sync, nc.scalar)[ci % 2]
                            dq.dma_start(
                                out_d[tb * 128:(tb + 1) * 128,
                                      n0:n0 + nw],
                                o_sb[:, n0:n0 + nw])
                            yield
                    if not (alt and p_ == 3):
                        dq = (nc.sync, nc.scalar)[tb % 2]
                        dq.dma_start(
                            out_d[tb * 128:(tb + 1) * 128, :], o_sb[:])
                        yield

            def vtail_steps():
                # deferred v-projection (tb 4-7, heads 8-9): near-pure-PE
                # filler for g0's ~10us of PE idle, which no o-proj can
                # reach by causality. Needed by g1's PV at h=8; consumed
                # fully within g0.
                for tb in range(4, 8):
                    ps = psum_o.tile([128, 512], f32, tag="ops",
                                     name=f"vt_{tb}")
                    for c in range(DC):
                        x0 = c * 512 + (tb % 4) * 128
                        nc.tensor.matmul(
                            ps[:, 0:256],
                            xt_q1[:, x0:x0 + 128],
                            wv3_t[:, c * 256:(c + 1) * 256],
                            start=(c == 0), stop=(c == DC - 1))
                        yield
                    for k in range(2):
                        base = tb * VW * H + (8 + k) * VW
                        nc.vector.tensor_copy(
                            vaug[:, base:base + 128],
                            ps[:, k * 128:(k + 1) * 128])
                        yield

            # group order 0..3: the no-filler first group is the SMALLEST
            # (20 pairs vs 80). Filler flows through a deque; each group
            # spreads the currently-available supply evenly across its
            # pairs (budget = avail), which also guarantees oproj(g) is
            # fully consumed within group g+1 (pool_yta bufs=2 WAR).
            fillers = deque()
            fillers.append(vtail_steps())
            fill_avail = 48
            fill_cap = {0: 48, 1: 10**6, 2: 10**6, 3: 10**6}
            for g in (0, 1, 2, 3):
                ytall = pool_yta.tile([128, 4 * D], bf16, tag="ytall")
                pair_i, fill_done = 0, 0
                total_pairs = H * (2 * g + 2)
                # budget = available supply clipped to the region's ACT
                # slack; for g3 take everything and spread it EVENLY
                # (front-loading a too-big budget exhausts supply at
                # ~pair 51/80 and the late heads stall unfilled)
                budget = min(fill_cap[g], fill_avail)
                for h in range(H):
                    # per-head y tile: rescales write it, the transposes
                    # at head end read it — no cross-head lifetime, so
                    # [128,512] x2 bufs instead of [128,4*D] x2 (18KB of
                    # SBUF freed to let pool_yta go to 3 bufs)
                    y_g = pool_y.tile([128, 512], bf16, tag="yg")
                    # two PV accumulators share each PSUM bank: r at col 0
                    # and r+1 at col 132. Only the col-0 chain's first
                    # matmul carries start=True (bank-wide has_written
                    # clear); the col-132 chain's first matmul relies on
                    # the cleared bits to overwrite, and an explicit dep
                    # keeps it ordered after the clearing matmul.
                    pva = psum_pv.tile([128, 512], f32, tag="pv",
                                       name=f"pva_{g}_{h}")
                    pvb = psum_pv.tile([128, 512], f32, tag="pv",
                                       name=f"pvb_{g}_{h}")
                    slots = [(pva, 0), (pva, 132), (pvb, 0), (pvb, 132)]
                    bank_clear_mm = {}
                    # strips in pairs sharing one [128,1024] psum tile
                    # (2 banks) -> one wide exp ACTIVATE per pair.
                    # Strips are PACKED (strip 1 lands right after strip
                    # 0's useful cols) so the exp covers useful columns
                    # only — the old fixed-512 layout exp'd up to 384
                    # dead gap cols on the two diagonal pairs of every
                    # (g,h), ~21us of ACT across the kernel, and ACT is
                    # the attention phase's co-bottleneck.
                    for m in range(2 * g + 2):
                        ps = psum_s.tile([128, 1024], f32, tag="ps")
                        pt = pool_pt.tile([128, 1024], bf16, tag="pt")
                        lo_pair, base_pair = [], []
                        off = 0
                        for idx in range(2):
                            j = 2 * m + idx
                            lo = max(0, 128 * j - 512 * g)
                            lo_pair.append(lo)
                            base_pair.append(off)
                            # packed: strip 1 may land mid-bank; its
                            # start=True re-clears the bank's has_written
                            # bits but preserves strip 0's data (nothing
                            # accumulates into those cols afterward)
                            nc.tensor.matmul(
                                ps[:, off:off + 512 - lo],
                                kt[:, h * T + j * 128:
                                   h * T + (j + 1) * 128],
                                qt[:, h * T + 512 * g + lo:
                                   h * T + 512 * (g + 1)],
                                start=True, stop=True,
                                skip_group_check=(idx == 1 and off < 512))
                            off += 512 - lo
                        nc.scalar.activation(
                            pt[:, 0:off], ps[:, 0:off],
                            EXP, scale=SCALE)
                        for idx in range(2):
                            j = 2 * m + idx
                            lo = lo_pair[idx]
                            cb = base_pair[idx]
                            if j >= 4 * g:
                                # mask stays on DVE: gpsimd's higher
                                # per-op latency on the exp->mask->PV
                                # critical path (the masked matmul is
                                # each slot's chain-closing stop=True op)
                                # measured +10us across g0/g1
                                nc.vector.tensor_mul(
                                    pt[:, cb:cb + 128],
                                    pt[:, cb:cb + 128],
                                    mask_t[:])
                            for p_ in range(4):
                                r = 4 * g + p_
                                if r < j:
                                    continue
                                tile_pv, off_pv = slots[p_]
                                c0 = cb + 128 * p_ - lo
                                mm = nc.tensor.matmul(
                                    tile_pv[:, off_pv:off_pv + VW],
                                    pt[:, c0:c0 + 128],
                                    vaug[:, j * VW * H + h * VW:
                                         j * VW * H + (h + 1) * VW],
                                    start=(j == 0 and off_pv == 0),
                                    stop=(j == r), skip_group_check=True)
                                key = tile_pv.name
                                if j == 0 and off_pv == 0:
                                    bank_clear_mm[key] = mm
                                elif j == 0:
                                    add_dep_helper(
                                        mm.ins, bank_clear_mm[key].ins,
                                        sync=False,
                                        reason="pv bank-clear order")
                        # pace queued o-proj steps as PE filler for this
                        # pair's exp-wait stall; fractional even spread
                        # of this region's budget across its pairs
                        if fillers:
                            tgt = (pair_i + 1) * budget // total_pairs
                            while fill_done < tgt and fillers:
                                try:
                                    next(fillers[0])
                                    fill_done += 1
                                    fill_avail -= 1
                                except StopIteration:
                                    fillers.popleft()
                        pair_i += 1
                    # per pv tile: reciprocal then both scales, so pva's
                    # last reader finishes before pvb's work starts and
                    # the bank frees ~0.8us earlier for the next head's
                    # PV chain. Scales stay on DVE (ACT variant cost
                    # +26us ACT and hurt net).
                    for t_i, tile_pv in enumerate((pva, pvb)):
                        rec2 = pool_small.tile([128, 2], f32, tag="rec")
                        nc.vector.reciprocal(
                            rec2[:].rearrange("p (s c) -> p s c", s=2),
                            tile_pv[:, 128:392].rearrange(
                                "p (s c) -> p s c", s=2)[:, :, 0:1])
                        for s_i in range(2):
                            p_ = 2 * t_i + s_i
                            off = slots[p_][1]
                            nc.vector.tensor_scalar(
                                y_g[:, p_ * 128:(p_ + 1) * 128],
                                tile_pv[:, off:off + 128],
                                rec2[:, s_i:s_i + 1],
                                None, MUL)
                    # transpose this head's 4 query blocks now: the PE
                    # work lands in the head-boundary exp-wait stall, and
                    # the group's o-proj becomes pure matmul chains
                    ytp = psum_yt.tile([128, 512], bf16, tag="yt")
                    for p_ in range(4):
                        nc.tensor.transpose(
                            ytp[:, p_ * 128:(p_ + 1) * 128],
                            y_g[:, p_ * 128:(p_ + 1) * 128],
                            ident_t[:])
                    # thin groups (g<=1) are DVE-bound per head: use the
                    # scalar engine there, DVE where ACT is exp-saturated
                    ytall_dst = ytall.rearrange("p (q x) -> p q x", q=4) \
                        [:, :, h * 128:(h + 1) * 128]
                    ytp_src = ytp[:].rearrange("p (q x) -> p q x", q=4)
                    if g >= 2:
                        nc.vector.tensor_copy(ytall_dst, ytp_src)
                    else:
                        nc.scalar.copy(ytall_dst, ytp_src)

                if stage == "attn":
                    continue   # debug mode no longer dumps y (per-head)

                # queue this group's o-proj as filler for later regions
                fillers.append(oproj_steps(g, ytall, alt=(g == 3),
                                           dve_copies=(g in (1, 2))))
                fill_avail += 139 if g == 3 else 136

            while fillers:
                for _ in fillers.popleft():
                    pass

        if stage != "proj":
            phase23()

    nc.compile()
    return nc


def _prep_inputs(x, consciousness_vector, Wq, Wk, Wv, Wo, Wg, bg):
    """Build the 8 per-core input maps (host-side layout prep + bf16 cast)."""
    x = np.asarray(x, np.float32)
    cv = np.asarray(consciousness_vector, np.float32)
    Wq = np.asarray(Wq, np.float32)
    Wk = np.asarray(Wk, np.float32)
    Wv = np.asarray(Wv, np.float32)
    Wo = np.asarray(Wo, np.float32)
    Wg = np.asarray(Wg, np.float32)
    bg = np.asarray(bg, np.float32)

    # lhsT layout for q/k: wq_arr[p, eb*D + c*128 + m] = W[eb*128+m, c*128+p]
    def qk_layout(W):
        return np.ascontiguousarray(
            W.reshape(H, 128, DC, 128).transpose(3, 0, 2, 1)
            .reshape(128, DC * D).astype(BF16))

    # rhs layout for v: wv_arr[p, c*D + e] = W[e, c*128+p]
    def dchunk_layout(W):
        return np.ascontiguousarray(
            W.reshape(D, DC, 128).transpose(2, 1, 0)
            .reshape(128, DC * D).astype(BF16))

    gates = 1.0 / (1.0 + np.exp(-(cv @ Wg.T + bg)))           # [B, H] f32

    wq_arr = qk_layout(Wq)
    wk_arr = qk_layout(Wk)
    # wv: piece-contiguous layout — [p, piece(n0), c, e] so each e-range
    # DMA piece is one contiguous column span (fast, unlike strided)
    wv_base = Wv.reshape(D, DC, 128).transpose(2, 1, 0)       # [p, c, e]
    wv_arr = np.ascontiguousarray(np.concatenate(
        [wv_base[:, :, n0:n0 + nw].reshape(128, DC * nw)
         for n0, nw in ((0, 512), (512, 512), (1024, 256))],
        axis=1).astype(BF16))
    # wo is per-core: the sigmoid gate for head h scales Wo's d-columns
    # of head h (h == d-chunk in the [p, h*D + e'] layout)

    invf = (10000.0 ** (-np.arange(0, 64, dtype=np.float64) * 2.0 / HD))
    ang = np.outer(invf, np.arange(T, dtype=np.float64))      # [64, T]
    cos_arr = np.concatenate([np.cos(ang), np.cos(ang)], 0).astype(BF16)
    srot_arr = np.concatenate([-np.sin(ang), np.sin(ang)], 0).astype(BF16)

    ii = np.arange(128)
    mask_arr = (ii[None, :] >= ii[:, None]).astype(BF16)      # col >= row
    ident_arr = np.eye(128, dtype=BF16)

    gates = 1.0 / (1.0 + np.exp(-(cv @ Wg.T + bg)))           # [B, H] f32

    in_maps = []
    for b in range(NCORES):
        # [p, q, c, tl] quarter-major to match the kernel's xt layout
        xt_arr = np.ascontiguousarray(
            x[b].T.reshape(DC, 128, 4, 512).transpose(1, 2, 0, 3)
            .reshape(128, DC * T).astype(BF16))
        gate_vec = np.repeat(gates[b], HD).astype(np.float32)  # [D]
        wo_arr = dchunk_layout(Wo * gate_vec[None, :])
        in_maps.append({
            "xt": xt_arr, "wq": wq_arr, "wk": wk_arr, "wv": wv_arr,
            "wo": wo_arr, "cosr": cos_arr, "srot": srot_arr,
            "trimask": mask_arr, "ident": ident_arr,
        })
    return in_maps


def get_program():
    if "nc" not in _cache:
        _cache["nc"] = _build_program()
    return _cache["nc"]


def run_on_cores(in_maps):
    from concourse.bass_utils import run_bass_kernel_spmd
    nc = get_program()
    res = run_bass_kernel_spmd(nc, in_maps, list(range(NCORES)))
    return res.results


def kernel(x, consciousness_vector, Wq, Wk, Wv, Wo, Wg, bg):
    in_maps = _prep_inputs(x, consciousness_vector, Wq, Wk, Wv, Wo, Wg, bg)
    for _attempt in range(3):
        results = run_on_cores(in_maps)
        out = np.stack([results[b]["out"] for b in range(NCORES)],
                       axis=0).astype(np.float32)
        if np.isfinite(out).all():
            break
    return out



# revision 42
# speedup vs baseline: 1.0091x; 1.0091x over previous
"""Trainium2 Bass kernel for DimSpecializedAttention.

Problem: B=8, T=2048, D=1280, H=10 heads, head_dim=128.
  q/k/v = x @ W{q,k,v}.T ; RoPE(q, k) ; causal softmax(q k^T / sqrt(128));
  per-head sigmoid gate (from consciousness_vector) applied post-softmax;
  out = (att @ v) @ Wo.T

Sharding: data-parallel over batch — core b gets batch b (8 cores, B=8).

Per-core kernel design (all matmuls bf16 with fp32 PSUM accumulation):
  - projections computed in transposed layout qT/kT [e, t] so the head dim
    lands on partitions (contraction-ready for attention); v in [t, e]
    layout with a ones-column appended per head ("vaug", stride 129).
  - scores computed transposed: S^T[tk, tq] = kT_j^T @ qT, causal blocks
    only (tq >= 128*j), exp on ScalarE straight out of PSUM (no
    max-subtraction: scores are ~N(0,1), max << 80, fp32-safe).
  - PV uses P^T tiles as the stationary operand and [v_j | 1] as moving:
    out[tq, 0:128] = attention numerator, out[:, 128] = softmax
    denominator — one fused accumulation chain per 128-row query block.
  - rows scaled by gate_h / denom (DVE), written to y; per 512-query
    group the output projection (PE transpose of y + Wo matmuls) runs
    overlapped with the next group's attention.
"""

import numpy as np
import ml_dtypes

BF16 = ml_dtypes.bfloat16

B, T, D = 8, 2048, 1280
H, HD = 10, 128
NCORES = 8
DC = D // 128      # 10 d-chunks
TB = T // 128      # 16 t-blocks
NG = 4             # attention groups of 512 queries
SCALE = float(1.0 / np.sqrt(HD))
VW = HD + 1        # 129: v columns per head incl. ones column

_cache = {}


def _build_program():
    import os
    import concourse.bacc as bacc
    import concourse.mybir as mybir
    import concourse.tile as tile
    from concourse.tile_rust import add_dep_helper
    from contextlib import ExitStack
    from collections import deque

    stage = os.environ.get("KSTAGE", "full")  # debug: proj | attn | full

    f32 = mybir.dt.float32
    bf16 = mybir.dt.bfloat16
    MUL = mybir.AluOpType.mult
    EXP = mybir.ActivationFunctionType.Exp

    nc = bacc.Bacc("TRN2", target_bir_lowering=False, debug=False,
                   num_devices=NCORES)

    xt_d = nc.dram_tensor("xt", [128, DC * T], bf16, kind="ExternalInput")
    wq_d = nc.dram_tensor("wq", [128, DC * D], bf16, kind="ExternalInput")
    wk_d = nc.dram_tensor("wk", [128, DC * D], bf16, kind="ExternalInput")
    wv_d = nc.dram_tensor("wv", [128, DC * D], bf16, kind="ExternalInput")
    wo_d = nc.dram_tensor("wo", [128, H * D], bf16, kind="ExternalInput")
    cos_d = nc.dram_tensor("cosr", [128, T], bf16, kind="ExternalInput")
    srot_d = nc.dram_tensor("srot", [128, T], bf16, kind="ExternalInput")
    mask_d = nc.dram_tensor("trimask", [128, 128], bf16, kind="ExternalInput")
    ident_d = nc.dram_tensor("ident", [128, 128], bf16, kind="ExternalInput")
    # bf16 output: halves the out-DMA bytes (tail latency); host upcasts.
    out_d = nc.dram_tensor("out", [T, D], bf16, kind="ExternalOutput")

    with tile.TileContext(nc) as tc, ExitStack() as ctx:
        # ---- persistent pools -------------------------------------------
        pool_const = ctx.enter_context(tc.tile_pool(name="const", bufs=1))
        pool_qkv = ctx.enter_context(tc.tile_pool(name="qkv", bufs=1))
        # xt quarter 1 and wv's n0=1024 piece live in persistent pools:
        # the v-projection chains for (tb 4-7, heads 8-9) are DEFERRED
        # into group 0's attention as PE filler (g0 is ACT/DVE-bound
        # with ~10us of PE idle and no o-proj available by causality),
        # so their inputs must survive into phase 2.
        pool_xt1 = ctx.enter_context(tc.tile_pool(name="xtq1", bufs=1))
        pool_wv3 = ctx.enter_context(tc.tile_pool(name="wv3", bufs=1))

        mask_t = pool_const.tile([128, 128], bf16, tag="mask")
        ident_t = pool_const.tile([128, 128], bf16, tag="ident")

        qt = pool_qkv.tile([128, H * T], bf16, tag="qt")
        kt = pool_qkv.tile([128, H * T], bf16, tag="kt")
        vaug = pool_qkv.tile([128, TB * VW * H], bf16, tag="vaug")
        # eb=0's wq block preloaded on the sync ring: the rotating web
        # pool aliases wv's range, so its first DMA is WAR-gated behind
        # the whole v-projection — this tile isn't.
        web0_t = pool_qkv.tile([128, D], bf16, tag="web0")

        # ---- phase 1: projections ---------------------------------------
        with tc.tile_pool(name="xtp", bufs=1) as pool_xt, \
             tc.tile_pool(name="projtmp", bufs=6) as pool_ptmp, \
             tc.tile_pool(name="projpsum", bufs=7, space="PSUM") as psum_proj:

            # xt is laid out quarter-major: col = q*DC*512 + c*512 + tl
            # (t = 512q + tl). Each t-quarter is one contiguous column
            # range, so the four loads are full-rate AND the dependency
            # tracker sees disjoint ranges — the first v chains start as
            # soon as quarter 0 lands (~19us) instead of after the whole
            # 5MB transfer.
            QW = DC * 512
            # quarters 0,2,3 in a phase-1 tile; quarter 1 persistent
            # (the deferred v-tail chains read it during attention g0)
            xt_r = pool_xt.tile([128, 3 * QW], bf16, tag="xt")
            xt_q1 = pool_xt1.tile([128, QW], bf16, tag="xtq1")

            def xt_at(q, off, w):
                if q == 1:
                    return xt_q1[:, off:off + w]
                base = (0, None, QW, 2 * QW)[q]
                return xt_r[:, base + off:base + off + w]

            # eighth-granular pieces, split across BOTH DMA rings so
            # arrival (~1.9us each) stays ahead of the v-proj consumption
            # rate (~2.1us per t-block chain): pieces 0-3 here on sync;
            # pieces 4-7 ride the scalar ring, emitted after wv's first
            # piece in the v-proj section below.
            for p0, pw in ((0, QW // 4), (QW // 4, QW // 4),
                           (QW // 2, QW // 2)):
                nc.sync.dma_start(xt_r[:, p0:p0 + pw],
                                  xt_d[:, p0:p0 + pw])
            for p0, pw in ((QW, QW // 2), (3 * QW // 2, QW // 2)):
                nc.sync.dma_start(xt_q1[:, p0 - QW:p0 - QW + pw],
                                  xt_d[:, p0:p0 + pw])
            nc.sync.dma_start(web0_t[:], wq_d[:, 0:D])
            # mask/ident after the xt pieces: they aren't read until the
            # proj-end bridge / attention, and descriptor-issue time at
            # the queue head delays the first xt piece ~1.6us otherwise
            nc.sync.dma_start(mask_t[:], mask_d[:])
            nc.sync.dma_start(ident_t[:], ident_d[:])

            # HAM warm-up bridge: keep the PE busy with throwaway matmuls
            # while the x/wv DMAs are in flight, so the real chains start
            # at the full 2.4GHz clock instead of the cold 1.2GHz default.
            # Tiles come from the long-lived proj pools — a dedicated pool
            # would alias the wv tile's address range and stall its DMA
            # until the last warm-up matmul retires.
            wt = pool_ptmp.tile([128, 512], bf16, tag="t2", name="warm_t")
            nc.gpsimd.memset(wt[:], 0.0)
            wps = psum_proj.tile([128, 512], f32, tag="pp", name="warm_ps")
            for _ in range(16):
                nc.tensor.matmul(wps[:], wt[:, 0:128], wt[:],
                                 start=True, stop=True,
                                 skip_group_check=True)
            # extend HAM-warm coverage deeper into the xt DMA wait with
            # N=256 warmups: first chains gate on DMA at ~21us, so the
            # bridge must reach there (trimming to 12+6 left a PE gap
            # and cost more than the throwaway work saved)
            for _ in range(8):
                nc.tensor.matmul(wps[:, 0:256], wt[:, 0:128],
                                 wt[:, 0:256], start=True, stop=True,
                                 skip_group_check=True)

            # vaug ones: memset ONLY each head's 129th column (160 elems
            # per partition, strided). The old full-tile memset took 18us
            # and its WAW dep stalled the v-proj copies ~6us at startup.
            nc.gpsimd.memset(
                vaug[:].rearrange("p (tb h c) -> p tb h c",
                                  tb=TB, h=H)[:, :, :, 128:129], 1.0)

            # v projection into vaug (head-interleaved, stride VW) — first,
            # so attention's PV inputs are ready as early as possible.
            with tc.tile_pool(name="wvp", bufs=1) as pool_wv:
                wv_t = pool_wv.tile([128, DC * D], bf16, tag="wv")
                # wv is pre-permuted host-side so each e-range piece is a
                # CONTIGUOUS column range: strided-piece DMAs ran at only
                # ~37GB/s and hogged the scalar ring for ~100us, starving
                # web/wo behind them (5.3us q/k stall + 6.7us boundary
                # stall while the wo gate-fold waited).
                # First piece in halves (finer splits shift every later
                # scalar-ring descriptor and cost more than they save)
                for s0, s1 in ((0, DC * 256), (DC * 256, DC * 512)):
                    nc.scalar.dma_start(wv_t[:, s0:s1], wv_d[:, s0:s1])
                # xt pieces 4-7 on the scalar ring, behind wv's first
                # piece (needed first) but ahead of its later ones
                for piece in range(4, 8):
                    p0 = piece * (QW // 2)
                    csl = slice(p0, p0 + QW // 2)
                    nc.scalar.dma_start(
                        xt_r[:, p0 - QW:p0 - QW + QW // 2], xt_d[:, csl])
                nc.scalar.dma_start(wv_t[:, DC * 512:2 * DC * 512],
                                    wv_d[:, DC * 512:2 * DC * 512])
                wv3_t = pool_wv3.tile([128, DC * 256], bf16, tag="wv3")
                nc.scalar.dma_start(
                    wv3_t[:], wv_d[:, 2 * DC * 512:2 * DC * 512 + DC * 256])
                for n0, nw in ((0, 512), (512, 512), (1024, 256)):
                    for tb in range(TB):
                        if n0 == 1024 and 4 <= tb < 8:
                            continue   # deferred: g0-attention filler
                        ps = psum_proj.tile([128, 512], f32, tag="pp")
                        for c in range(DC):
                            wsrc = (wv3_t[:, c * nw:(c + 1) * nw]
                                    if n0 == 1024 else
                                    wv_t[:, (0 if n0 == 0 else DC * 512)
                                         + c * nw:
                                         (0 if n0 == 0 else DC * 512)
                                         + (c + 1) * nw])
                            nc.tensor.matmul(
                                ps[:, 0:nw],
                                xt_at(tb // 4,
                                      c * 512 + (tb % 4) * 128, 128),
                                wsrc,
                                start=(c == 0), stop=(c == DC - 1))
                        for k in range(nw // 128):
                            h = (n0 + k * 128) // 128
                            base = tb * VW * H + h * VW
                            nc.vector.tensor_copy(
                                vaug[:, base:base + 128],
                                ps[:, k * 128:(k + 1) * 128])

            # q/k projections (transposed out, + RoPE)
            with tc.tile_pool(name="web", bufs=2) as pool_web, \
                 tc.tile_pool(name="rope", bufs=1) as pool_rope:
                cos_t = pool_rope.tile([128, T], bf16, tag="cos")
                srot_t = pool_rope.tile([128, T], bf16, tag="srot")
                # sync ring: transfers in parallel with web0 on the
                # scalar ring once the wv-range WAR releases
                nc.sync.dma_start(cos_t[:], cos_d[:])
                nc.sync.dma_start(srot_t[:], srot_d[:])

                for eb in range(H):
                    for w_d, dst in ((wq_d, qt), (wk_d, kt)):
                        if eb == 0 and w_d is wq_d:
                            web = web0_t     # preloaded on the sync ring
                        else:
                            web = pool_web.tile([128, D], bf16, tag="web")
                            nc.scalar.dma_start(
                                web[:], w_d[:, eb * D:(eb + 1) * D])
                        for tcn in range(T // 512):
                            ps = psum_proj.tile([128, 512], f32, tag="pp")
                            for c in range(DC):
                                nc.tensor.matmul(
                                    ps[:],
                                    web[:, c * 128:(c + 1) * 128],
                                    xt_at(tcn, c * 512, 512),
                                    start=(c == 0), stop=(c == DC - 1))
                            # RoPE: ScalarE stages the partition-rotated
                            # copy from PSUM (ACT is idle in this phase),
                            # then DVE does mul/mul/add.
                            qrot = pool_ptmp.tile([128, 512], bf16,
                                                  tag="qrot")
                            nc.scalar.copy(qrot[0:64, :], ps[64:128, :])
                            nc.scalar.copy(qrot[64:128, :], ps[0:64, :])
                            t2 = pool_ptmp.tile([128, 512], bf16, tag="t2")
                            sl = slice(tcn * 512, (tcn + 1) * 512)
                            o = dst[:, eb * T + tcn * 512:
                                    eb * T + (tcn + 1) * 512]
                            nc.vector.tensor_mul(t2[:], qrot[:],
                                                 srot_t[:, sl])
                            nc.vector.tensor_mul(o, ps[:], cos_t[:, sl])
                            nc.vector.tensor_add(o, o, t2[:])

            wps_c = psum_proj.tile([128, 512], f32, tag="pp",
                                   name="bridge2_ps")
            for _ in range(8):
                nc.tensor.matmul(wps_c[:], ident_t[:], xt_r[:, 0:512],
                                 start=True, stop=True,
                                 skip_group_check=True)

        if stage == "proj":
            # debug: dump slices of qt/kt/vaug into out rows
            with tc.tile_pool(name="dbg", bufs=2) as pool_dbg:
                for nm, src in (("q", qt), ("k", kt), ("v", vaug)):
                    od = pool_dbg.tile([128, 1024], bf16, tag="od",
                                       name=f"od_{nm}")
                    nc.scalar.copy(od[:], src[:, 0:1024])
                    row = {"q": 0, "k": 128, "v": 256}[nm]
                    nc.sync.dma_start(out_d[row:row + 128, 0:1024], od[:])

        # ---- phase 2+3: attention + output projection -------------------
        def phase23():
          with tc.tile_pool(name="pt", bufs=6) as pool_pt, \
             tc.tile_pool(name="yg", bufs=2) as pool_y, \
             tc.tile_pool(name="yta", bufs=2) as pool_yta, \
             tc.tile_pool(name="osb", bufs=2) as pool_osb, \
             tc.tile_pool(name="wop", bufs=1) as pool_wo, \
             tc.tile_pool(name="small", bufs=8) as pool_small, \
             tc.tile_pool(name="spsum", bufs=2, space="PSUM") as psum_s, \
             tc.tile_pool(name="pvpsum", bufs=2, space="PSUM") as psum_pv, \
             tc.tile_pool(name="ytpsum", bufs=1, space="PSUM") as psum_yt, \
             tc.tile_pool(name="opsum", bufs=1, space="PSUM") as psum_o:

            # wo on the otherwise-idle SYNC ring (the scalar ring still
            # has web DMAs queued ahead at phase-2 entry, which delayed
            # wo to ~300us and stalled the first o-proj filler 6.6us),
            # in per-head pieces so the first o-proj chain (which reads
            # only head 0's columns first) can start on piece 0.
            wo_t = pool_wo.tile([128, H * D], bf16, tag="wo")
            for h_ in range(H):
                nc.sync.dma_start(wo_t[:, h_ * D:(h_ + 1) * D],
                                  wo_d[:, h_ * D:(h_ + 1) * D])

            def oproj_steps(g, ytall_g, alt=False, dve_copies=False):
                """Emit group g's output projection one instruction per
                yield, so the caller can pace it into the next group's
                strip-pair loop as PE filler for the exp-wait stalls.
                alt=True (final drain only, when the transpose PSUM bank
                is idle) alternates chains across two banks so each
                chain overlaps the previous chain's drain copy.
                dve_copies=True routes all drain copies to DVE — for
                o-proj paced into g2/g3, whose regions are ACT-bound
                (exp-saturated) while DVE has ~60% slack."""
                for p_ in range(4):
                    tb = 4 * g + p_
                    o_sb = pool_osb.tile([128, D], bf16, tag="osb")
                    for ci, (n0, nw) in enumerate(
                            ((0, 512), (512, 512), (1024, 256))):
                        k = p_ * 3 + ci
                        if alt:
                            # final drain: rotate chains over FOUR psum
                            # banks (o, yt, pva, pvb — attention is done,
                            # all free) so each chain's drain copy has 3
                            # chain-times to complete instead of 1
                            if k % 4 == 0:
                                ops = psum_o.tile([128, 512], f32,
                                                  tag="ops")
                            elif k % 4 == 1:
                                ops = psum_yt.tile([128, 512], f32,
                                                   tag="yt")
                            else:
                                ops = psum_pv.tile([128, 512], f32,
                                                   tag="pv")
                        else:
                            ops = psum_o.tile([128, 512], f32, tag="ops")
                        for h in range(H):
                            nc.tensor.matmul(
                                ops[:, 0:nw],
                                ytall_g[:, p_ * D + h * 128:
                                        p_ * D + (h + 1) * 128],
                                wo_t[:, h * D + n0:h * D + n0 + nw],
                                start=(h == 0), stop=(h == H - 1))
                            yield
                        use_scalar = ((k % 2 == 0) if alt
                                      else (ci == 1 and not dve_copies))
                        if use_scalar:
                            nc.scalar.copy(o_sb[:, n0:n0 + nw],
                                           ops[:, 0:nw])
                        else:
                            nc.vector.tensor_copy(o_sb[:, n0:n0 + nw],
                                                  ops[:, 0:nw])
                        yield
                        if alt and p_ == 3:
                            # last block: per-piece DMA right after each
                            # copy, so the final transfer tail is one
                            # 256-col piece instead of the whole block
                            dq = (nc.sync, nc.scalar)[ci % 2]
                            dq.dma_start(
                                out_d[tb * 128:(tb + 1) * 128,
                                      n0:n0 + nw],
                                o_sb[:, n0:n0 + nw])
                            yield
                    if not (alt and p_ == 3):
                        dq = (nc.sync, nc.scalar)[tb % 2]
                        dq.dma_start(
                            out_d[tb * 128:(tb + 1) * 128, :], o_sb[:])
                        yield

            def vtail_steps():
                # deferred v-projection (tb 4-7, heads 8-9): near-pure-PE
                # filler for g0's ~10us of PE idle, which no o-proj can
                # reach by causality. Needed by g1's PV at h=8; consumed
                # fully within g0.
                for tb in range(4, 8):
                    ps = psum_o.tile([128, 512], f32, tag="ops",
                                     name=f"vt_{tb}")
                    for c in range(DC):
                        x0 = c * 512 + (tb % 4) * 128
                        nc.tensor.matmul(
                            ps[:, 0:256],
                            xt_q1[:, x0:x0 + 128],
                            wv3_t[:, c * 256:(c + 1) * 256],
                            start=(c == 0), stop=(c == DC - 1))
                        yield
                    for k in range(2):
                        base = tb * VW * H + (8 + k) * VW
                        nc.vector.tensor_copy(
                            vaug[:, base:base + 128],
                            ps[:, k * 128:(k + 1) * 128])
                        yield

            # group order 0..3: the no-filler first group is the SMALLEST
            # (20 pairs vs 80). Filler flows through a deque; each group
            # spreads the currently-available supply evenly across its
            # pairs (budget = avail), which also guarantees oproj(g) is
            # fully consumed within group g+1 (pool_yta bufs=2 WAR).
            fillers = deque()
            fillers.append(vtail_steps())
            fill_avail = 48
            fill_cap = {0: 48, 1: 10**6, 2: 10**6, 3: 10**6}
            for g in (0, 1, 2, 3):
                ytall = pool_yta.tile([128, 4 * D], bf16, tag="ytall")
                pair_i, fill_done = 0, 0
                total_pairs = H * (2 * g + 2)
                # budget = available supply clipped to the region's ACT
                # slack; for g3 take everything and spread it EVENLY
                # (front-loading a too-big budget exhausts supply at
                # ~pair 51/80 and the late heads stall unfilled)
                budget = min(fill_cap[g], fill_avail)
                for h in range(H):
                    # per-head y tile: rescales write it, the transposes
                    # at head end read it — no cross-head lifetime, so
                    # [128,512] x2 bufs instead of [128,4*D] x2 (18KB of
                    # SBUF freed to let pool_yta go to 3 bufs)
                    y_g = pool_y.tile([128, 512], bf16, tag="yg")
                    # two PV accumulators share each PSUM bank: r at col 0
                    # and r+1 at col 132. Only the col-0 chain's first
                    # matmul carries start=True (bank-wide has_written
                    # clear); the col-132 chain's first matmul relies on
                    # the cleared bits to overwrite, and an explicit dep
                    # keeps it ordered after the clearing matmul.
                    pva = psum_pv.tile([128, 512], f32, tag="pv",
                                       name=f"pva_{g}_{h}")
                    pvb = psum_pv.tile([128, 512], f32, tag="pv",
                                       name=f"pvb_{g}_{h}")
                    slots = [(pva, 0), (pva, 132), (pvb, 0), (pvb, 132)]
                    bank_clear_mm = {}
                    # strips in pairs sharing one [128,1024] psum tile
                    # (2 banks) -> one wide exp ACTIVATE per pair.
                    # Strips are PACKED (strip 1 lands right after strip
                    # 0's useful cols) so the exp covers useful columns
                    # only — the old fixed-512 layout exp'd up to 384
                    # dead gap cols on the two diagonal pairs of every
                    # (g,h), ~21us of ACT across the kernel, and ACT is
                    # the attention phase's co-bottleneck.
                    for m in range(2 * g + 2):
                        ps = psum_s.tile([128, 1024], f32, tag="ps")
                        pt = pool_pt.tile([128, 1024], bf16, tag="pt")
                        lo_pair, base_pair = [], []
                        off = 0
                        for idx in range(2):
                            j = 2 * m + idx
                            lo = max(0, 128 * j - 512 * g)
                            lo_pair.append(lo)
                            base_pair.append(off)
                            # packed: strip 1 may land mid-bank; its
                            # start=True re-clears the bank's has_written
                            # bits but preserves strip 0's data (nothing
                            # accumulates into those cols afterward)
                            nc.tensor.matmul(
                                ps[:, off:off + 512 - lo],
                                kt[:, h * T + j * 128:
                                   h * T + (j + 1) * 128],
                                qt[:, h * T + 512 * g + lo:
                                   h * T + 512 * (g + 1)],
                                start=True, stop=True,
                                skip_group_check=(idx == 1 and off < 512))
                            off += 512 - lo
                        nc.scalar.activation(
                            pt[:, 0:off], ps[:, 0:off],
                            EXP, scale=SCALE)
                        for idx in range(2):
                            j = 2 * m + idx
                            lo = lo_pair[idx]
                            cb = base_pair[idx]
                            if j >= 4 * g:
                                # mask stays on DVE: gpsimd's higher
                                # per-op latency on the exp->mask->PV
                                # critical path (the masked matmul is
                                # each slot's chain-closing stop=True op)
                                # measured +10us across g0/g1
                                nc.vector.tensor_mul(
                                    pt[:, cb:cb + 128],
                                    pt[:, cb:cb + 128],
                                    mask_t[:])
                            for p_ in range(4):
                                r = 4 * g + p_
                                if r < j:
                                    continue
                                tile_pv, off_pv = slots[p_]
                                c0 = cb + 128 * p_ - lo
                                mm = nc.tensor.matmul(
                                    tile_pv[:, off_pv:off_pv + VW],
                                    pt[:, c0:c0 + 128],
                                    vaug[:, j * VW * H + h * VW:
                                         j * VW * H + (h + 1) * VW],
                                    start=(j == 0 and off_pv == 0),
                                    stop=(j == r), skip_group_check=True)
                                key = tile_pv.name
                                if j == 0 and off_pv == 0:
                                    bank_clear_mm[key] = mm
                                elif j == 0:
                                    add_dep_helper(
                                        mm.ins, bank_clear_mm[key].ins,
                                        sync=False,
                                        reason="pv bank-clear order")
                        # pace queued o-proj steps as PE filler for this
                        # pair's exp-wait stall; fractional even spread
                        # of this region's budget across its pairs
                        if fillers:
                            tgt = (pair_i + 1) * budget // total_pairs
                            while fill_done < tgt and fillers:
                                try:
                                    next(fillers[0])
                                    fill_done += 1
                                    fill_avail -= 1
                                except StopIteration:
                                    fillers.popleft()
                        pair_i += 1
                    # per pv tile: reciprocal then both scales, so pva's
                    # last reader finishes before pvb's work starts and
                    # the bank frees ~0.8us earlier for the next head's
                    # PV chain. Scales stay on DVE (ACT variant cost
                    # +26us ACT and hurt net).
                    for t_i, tile_pv in enumerate((pva, pvb)):
                        rec2 = pool_small.tile([128, 2], f32, tag="rec")
                        nc.vector.reciprocal(
                            rec2[:].rearrange("p (s c) -> p s c", s=2),
                            tile_pv[:, 128:392].rearrange(
                                "p (s c) -> p s c", s=2)[:, :, 0:1])
                        for s_i in range(2):
                            p_ = 2 * t_i + s_i
                            off = slots[p_][1]
                            nc.vector.tensor_scalar(
                                y_g[:, p_ * 128:(p_ + 1) * 128],
                                tile_pv[:, off:off + 128],
                                rec2[:, s_i:s_i + 1],
                                None, MUL)
                    # transpose this head's 4 query blocks now: the PE
                    # work lands in the head-boundary exp-wait stall, and
                    # the group's o-proj becomes pure matmul chains
                    ytp = psum_yt.tile([128, 512], bf16, tag="yt")
                    for p_ in range(4):
                        nc.tensor.transpose(
                            ytp[:, p_ * 128:(p_ + 1) * 128],
                            y_g[:, p_ * 128:(p_ + 1) * 128],
                            ident_t[:])
                    # thin groups (g<=1) are DVE-bound per head: use the
                    # scalar engine there, DVE where ACT is exp-saturated
                    ytall_dst = ytall.rearrange("p (q x) -> p q x", q=4) \
                        [:, :, h * 128:(h + 1) * 128]
                    ytp_src = ytp[:].rearrange("p (q x) -> p q x", q=4)
                    if g >= 2:
                        nc.vector.tensor_copy(ytall_dst, ytp_src)
                    else:
                        nc.scalar.copy(ytall_dst, ytp_src)

                if stage == "attn":
                    continue   # debug mode no longer dumps y (per-head)

                # queue this group's o-proj as filler for later regions
                fillers.append(oproj_steps(g, ytall, alt=(g == 3),
                                           dve_copies=(g in (1, 2))))
                fill_avail += 139 if g == 3 else 136

            while fillers:
                for _ in fillers.popleft():
                    pass

        if stage != "proj":
            phase23()

    nc.compile()
    return nc


def _prep_inputs(x, consciousness_vector, Wq, Wk, Wv, Wo, Wg, bg):
    """Build the 8 per-core input maps (host-side layout prep + bf16 cast)."""
    x = np.asarray(x, np.float32)
    cv = np.asarray(consciousness_vector, np.float32)
    Wq = np.asarray(Wq, np.float32)
    Wk = np.asarray(Wk, np.float32)
    Wv = np.asarray(Wv, np.float32)
    Wo = np.asarray(Wo, np.float32)
    Wg = np.asarray(Wg, np.float32)
    bg = np.asarray(bg, np.float32)

    # lhsT layout for q/k: wq_arr[p, eb*D + c*128 + m] = W[eb*128+m, c*128+p]
    def qk_layout(W):
        return np.ascontiguousarray(
            W.reshape(H, 128, DC, 128).transpose(3, 0, 2, 1)
            .reshape(128, DC * D).astype(BF16))

    # rhs layout for v: wv_arr[p, c*D + e] = W[e, c*128+p]
    def dchunk_layout(W):
        return np.ascontiguousarray(
            W.reshape(D, DC, 128).transpose(2, 1, 0)
            .reshape(128, DC * D).astype(BF16))

    gates = 1.0 / (1.0 + np.exp(-(cv @ Wg.T + bg)))           # [B, H] f32

    wq_arr = qk_layout(Wq)
    wk_arr = qk_layout(Wk)
    # wv: piece-contiguous layout — [p, piece(n0), c, e] so each e-range
    # DMA piece is one contiguous column span (fast, unlike strided)
    wv_base = Wv.reshape(D, DC, 128).transpose(2, 1, 0)       # [p, c, e]
    wv_arr = np.ascontiguousarray(np.concatenate(
        [wv_base[:, :, n0:n0 + nw].reshape(128, DC * nw)
         for n0, nw in ((0, 512), (512, 512), (1024, 256))],
        axis=1).astype(BF16))
    # wo is per-core: the sigmoid gate for head h scales Wo's d-columns
    # of head h (h == d-chunk in the [p, h*D + e'] layout)

    invf = (10000.0 ** (-np.arange(0, 64, dtype=np.float64) * 2.0 / HD))
    ang = np.outer(invf, np.arange(T, dtype=np.float64))      # [64, T]
    cos_arr = np.concatenate([np.cos(ang), np.cos(ang)], 0).astype(BF16)
    srot_arr = np.concatenate([-np.sin(ang), np.sin(ang)], 0).astype(BF16)

    ii = np.arange(128)
    mask_arr = (ii[None, :] >= ii[:, None]).astype(BF16)      # col >= row
    ident_arr = np.eye(128, dtype=BF16)

    gates = 1.0 / (1.0 + np.exp(-(cv @ Wg.T + bg)))           # [B, H] f32

    in_maps = []
    for b in range(NCORES):
        # [p, q, c, tl] quarter-major to match the kernel's xt layout
        xt_arr = np.ascontiguousarray(
            x[b].T.reshape(DC, 128, 4, 512).transpose(1, 2, 0, 3)
            .reshape(128, DC * T).astype(BF16))
        gate_vec = np.repeat(gates[b], HD).astype(np.float32)  # [D]
        wo_arr = dchunk_layout(Wo * gate_vec[None, :])
        in_maps.append({
            "xt": xt_arr, "wq": wq_arr, "wk": wk_arr, "wv": wv_arr,
            "wo": wo_arr, "cosr": cos_arr, "srot": srot_arr,
            "trimask": mask_arr, "ident": ident_arr,
        })
    return in_maps


def get_program():
    if "nc" not in _cache:
        _cache["nc"] = _build_program()
    return _cache["nc"]


def run_on_cores(in_maps):
    from concourse.bass_utils import run_bass_kernel_spmd
    nc = get_program()
    res = run_bass_kernel_spmd(nc, in_maps, list(range(NCORES)))
    return res.results


def kernel(x, consciousness_vector, Wq, Wk, Wv, Wo, Wg, bg):
    in_maps = _prep_inputs(x, consciousness_vector, Wq, Wk, Wv, Wo, Wg, bg)
    for _attempt in range(3):
        results = run_on_cores(in_maps)
        out = np.stack([results[b]["out"] for b in range(NCORES)],
                       axis=0).astype(np.float32)
        if np.isfinite(out).all():
            break
    return out

